# revision 47
# baseline (speedup 1.0000x reference)
"""Complex 3D+temporal conv (ComplexPadConv3Dt) on 8 Trainium2 NeuronCores.

Strategy (hardcoded for B=2, T=8, Z=20, Y=64, X=64, C=2, F1=F=32, k=3):
 - Pure data-parallel sharding: 8 cores = B(2) x X-quarters(4). Each core
   computes its (b, 16-wide x slab) including halo; no collectives.
 - All matmuls bf16 (rel err ~5e-3 vs the 2e-2 gate), PSUM accumulates f32.
 - The PE overlaps a 4-matmul quadrant wave fully (~216ns, the N=512
   streaming time) only when the two tiles in each column-half stream the
   SAME rhs address into both partition halves. Both phases are built
   around such waves:
   * Spatial conv: K=36 contraction (dz,dy)x(c,ri), dz/dy baked into the
     DRAM relayout, dx as a free-dim x offset (3 accumulating waves).
     SBUF slab partitions 0-35 hold even-z (z,j)-addressed data;
     partitions 64-99 hold the odd-z data at the same addresses, so one
     (zp,j) address feeds z=even from the low row half and z=odd from
     the high row half. Only even-z addresses are ever streamed, so the
     main load ships just the even-z rows (halves main input DMA).
   * Per (t, z-pair) outputs land in a [128,1024] 2-bank PSUM tile; the
     bf16 slices copy is slot0 = [(ze,j0) lo; (zo,j1) hi], slot1 =
     [(zo,j0) lo; (ze,j1) hi].
   * Temporal conv: K=64 contraction (q,f1), 3 taps accumulated; col
     half = j address slot, row half = z parity; bank ze comes out
     straight [(ze,j0); (ze,j1)], bank zo j-swapped (host undoes it).
 - Evacuations split each 2-bank PSUM tile across BOTH psum-capable
   engines (ScalarE bank A, DVE bank B, alternating) so the tile frees
   in ~0.66us and the 4-deep psum rotation keeps the PE fed.
 - DMA layout tuned for the SDMA engines' ~55ns/packet overhead and
   DRAM-page splits: every stream moves in 4KB-aligned pieces.
   * Inputs: per (row, z-block) the whole 8-t run (36864B) is contiguous
     in DRAM, padded to a 40960B slot -> nine perfect 4KB packets. One
     main DMA (rows 0-35, even SDMA engines, ScalarE ring) and one
     swapped-copy DMA (rows 64-99, odd engines, GpSimd SWDGE ring) per
     z-block: the two halves load in parallel on disjoint engines.
   * Temporal results accumulate per (t, z-block) into a [128, 2048]
     bf16 tile, DMA'd as ONE ~0.5MB transfer with 4KB-aligned
     per-partition runs into outq[T, 128, Z, 512] (host un-permutes).
 - temporal(t) issues after spatial(t+2): output DMAs/evacs spread
   evenly and temporal's last tap never chases a just-written slice.
 - ~3.5us of dummy matmuls at startup open the PE HAM clock gate to
   2.4GHz before the first real wave.
 - Outputs stored bf16, upcast on host.
"""

import numpy as np
import ml_dtypes

import concourse.bass as bass
import concourse.bacc as bacc
import concourse.mybir as mybir
from concourse import tile
from concourse.bass_utils import run_bass_kernel_spmd

# Problem constants
B, T, Z, Y, X, C = 2, 8, 20, 64, 64, 2
F1, F = 32, 32
KZ = KY = KX = 3
KT = 3

# Sharding / tiling
XC = 16          # output x columns per core
NXC = X // XC    # 4 x-chunks
XI = XC + 2      # input x columns per core (halo)
ZB = 4           # z rows per block
NZB = Z // ZB    # 5 blocks
NR = 36          # spatial contraction rows (dz,dy,c,ri)
RW = 2 * XI * 32          # 1152 elems per (z, t) row
ZROW = T * 2 * RW         # 18432 elems per (r, zb): 8t x 2zp x 1152
NPC = 9                   # ZROW = nine 2048-elem (4KB = one DRAM page) pieces
# each piece sits in a 4096-elem (8KB) slot: the non-contiguous stride
# stops bass's AP optimizer from re-merging pieces into one big run, so
# every DMA descriptor is exactly one aligned DRAM page (25.8GB/s/engine
# measured, vs 13.3 for a merged 36KB run that crosses 8 page boundaries)

F32 = mybir.dt.float32
BF16 = mybir.dt.bfloat16
BF16NP = ml_dtypes.bfloat16

_NC_CACHE = {}


def _project(wr, wi, zero_mean):
    wr = wr.astype(np.float64)
    wi = wi.astype(np.float64)
    ax = (0, 1, 2, 3)
    if zero_mean:
        wr = wr - wr.mean(ax, keepdims=True)
        wi = wi - wi.mean(ax, keepdims=True)
    norm = np.sqrt((wr * wr + wi * wi).sum(ax, keepdims=True))
    s = 1.0 / np.maximum(norm, 1.0)
    return wr * s, wi * s


def _spatial_lhsT(wsr, wsi):
    """[128, 3*64] bf16. Col block dx; rows r = (dz*3+dy)*4 + c*2 + ri at
    partitions 0-35 and duplicated at 64-99. Cols: q'*32 + f."""
    w = np.zeros((128, 3 * 64), np.float64)
    for dx in range(KX):
        for dz in range(KZ):
            for dy in range(KY):
                for c in range(C):
                    r0 = (dz * 3 + dy) * 4 + c * 2
                    col = dx * 64
                    wr = wsr[dz, dy, dx, c, :]
                    wi = wsi[dz, dy, dx, c, :]
                    for base in (0, 64):
                        w[base + r0 + 0, col + 0:col + 32] = wr
                        w[base + r0 + 0, col + 32:col + 64] = wi
                        w[base + r0 + 1, col + 0:col + 32] = -wi
                        w[base + r0 + 1, col + 32:col + 64] = wr
    return w.astype(BF16NP)


def _temporal_lhsT(wtr, wti):
    """[128, 5*64] bf16. rows 64d + q*32 + f1 (q=0 spr, 1 spi); cols q'*32 + f.

    variants v: [wt0, wt1, wt2, wt0+wt1, wt1+wt2]
    """
    wtr = wtr.reshape(KT, F1, F)
    wti = wti.reshape(KT, F1, F)
    variants = [
        (wtr[0], wti[0]),
        (wtr[1], wti[1]),
        (wtr[2], wti[2]),
        (wtr[0] + wtr[1], wti[0] + wti[1]),
        (wtr[1] + wtr[2], wti[1] + wti[2]),
    ]
    w = np.zeros((64, 5 * 64), np.float64)
    for v, (vr, vi) in enumerate(variants):
        w[0:32, v * 64 + 0:v * 64 + 32] = vr          # spr -> yr
        w[0:32, v * 64 + 32:v * 64 + 64] = vi         # spr -> yi
        w[32:64, v * 64 + 0:v * 64 + 32] = -vi        # spi -> yr
        w[32:64, v * 64 + 32:v * 64 + 64] = vr        # spi -> yi
    out = np.zeros((128, 5 * 64), np.float64)
    out[0:64] = w
    out[64:128] = w
    return out.astype(BF16NP)


def _temporal_taps(t):
    if t == 0:
        return [(0, 3), (1, 2)]
    if t == T - 1:
        return [(T - 2, 0), (T - 1, 4)]
    return [(t - 1, 0), (t, 1), (t + 1, 2)]


def build_program():
    nc = bacc.Bacc(None, target_bir_lowering=False)

    # xev: even-z rows, xod: odd-z rows (the "z-swapped" copy). Per
    # (r, zb) one contiguous 36864B run in a 40960B (page-aligned) slot.
    xev = nc.declare_dram_parameter("xev", [NR, NZB, NPC, 4096], BF16, isOutput=False)
    xod = nc.declare_dram_parameter("xod", [NR, NZB, NPC, 4096], BF16, isOutput=False)
    # startup fast path: z-block 0 duplicated as four t-pair chunks so
    # the first waves launch after ~0.17MB instead of ~2.7MB of DMA
    xev0 = nc.declare_dram_parameter("xev0", [NR, 4, 2 * 2304], BF16, isOutput=False)
    xod0 = nc.declare_dram_parameter("xod0", [NR, 4, 2 * 2304], BF16, isOutput=False)
    wsp = nc.declare_dram_parameter("wsp", [128, 3 * 64], BF16, isOutput=False)
    wtp = nc.declare_dram_parameter("wtp", [128, 5 * 64], BF16, isOutput=False)
    outq = nc.declare_dram_parameter("outq", [T, 128, Z, 512], BF16, isOutput=True)

    with tile.TileContext(nc) as tc:
        with (
            tc.tile_pool(name="wpool", bufs=1) as wpool,
            tc.tile_pool(name="slabs", bufs=2) as slab_pool,
            tc.tile_pool(name="slab0", bufs=4) as slab0_pool,
            tc.tile_pool(name="slices", bufs=7) as slice_pool,
            tc.tile_pool(name="tmp", bufs=4) as tmp_pool,
            tc.tile_pool(name="psum", bufs=8, space="PSUM") as psum_pool,
        ):
            wsp_sb = wpool.tile([128, 3 * 64], BF16, name="wsp_sb", tag="wsp")
            wtp_sb = wpool.tile([128, 5 * 64], BF16, name="wtp_sb", tag="wtp")
            nc.sync.dma_start(out=wsp_sb[:], in_=wsp[:])
            nc.sync.dma_start(out=wtp_sb[:], in_=wtp[:])

            def load_slab(zb):
                # whole z-block slab [100, 8t x 2zp x 1152]; rows 0-35 =
                # even z, rows 64-99 = odd z at the same addresses. Two
                # DMAs on separate rings (ScalarE HWDGE / GpSimd SWDGE)
                # landing on disjoint (even/odd) SDMA engine sets.
                sl = slab_pool.tile([100, ZROW], BF16, name="sl", tag="sl")
                sl_k = sl.rearrange("p (k r) -> p k r", k=NPC)
                # one input stream per HWDGE ring, balancing ring load
                nc.scalar.dma_start(
                    out=sl_k[0:NR], in_=xev[:, zb, :, 0:2048]
                )
                nc.sync.dma_start(
                    out=sl_k[64:64 + NR], in_=xod[:, zb, :, 0:2048]
                )
                return sl.rearrange(
                    "p (t z j x y) -> p t z j x y", t=T, z=2, j=2, x=XI, y=32
                )

            def load_slab0(tp):
                sl = slab0_pool.tile([100, 2 * 2304], BF16, name="sl0", tag="sl0")
                nc.scalar.dma_start(out=sl[0:NR, :], in_=xev0[:, tp])
                nc.scalar.dma_start(out=sl[64:64 + NR, :], in_=xod0[:, tp])
                return sl.rearrange(
                    "p (t z j x y) -> p t z j x y", t=2, z=2, j=2, x=XI, y=32
                )

            slab0 = [load_slab0(tp) for tp in range(4)]

            # HAM warmup: ~4-6us of dummy matmuls (on resident weight
            # tiles, into the first psum slot, overwritten later by the
            # first real start=True wave) so the PE clock-gate opens to
            # 2.4GHz while the first slab loads.
            warm_ps = psum_pool.tile([128, 512], F32, name="wps", tag="ps")
            for _ in range(64):
                nc.tensor.matmul(
                    out=warm_ps[0:64, 0:192],
                    lhsT=wsp_sb[:, 0:64], rhs=wsp_sb[:, 0:192],
                    start=False, stop=False, tile_position=(0, 0),
                )

            def evac(dst, psb, flip):
                # whole single-bank tile to one engine, alternating: the
                # 8-deep psum rotation gives the loop latency ~2.3us of
                # budget, so neither the PE nor the evac engines ever
                # block on a slot return
                if flip:
                    nc.scalar.copy(dst, psb[:, :])
                else:
                    nc.vector.tensor_copy(dst, psb[:, :])

            next_slab = None
            for zb in range(NZB):
                z0 = zb * ZB
                if zb == 0:
                    def rhs_view(t):
                        return slab0[t // 2], t % 2
                else:
                    sl_whole = next_slab

                    def rhs_view(t, _s=sl_whole):
                        return _s, t

                # ---- spatial phase ----
                # Per (t, z-pair): [128,1024]: bank A (free 0-511) =
                # [(ze,j0); (zo,j1)], bank B = [(zo,j0) lo; (ze,j1) hi].
                # Wave: col half = j address; row half lo = ze data, hi =
                # zo data (odd-z rows); same col half streams one address.
                slices = [None] * T

                def spatial(t):
                    slc = slice_pool.tile([128, ZB * 512], BF16, name="slc", tag="slc")
                    slices[t] = slc
                    sl_v, tv = rhs_view(t)
                    for zp in range(ZB // 2):
                        psa = psum_pool.tile([128, 512], F32, name="ps", tag="ps")
                        psb = psum_pool.tile([128, 512], F32, name="ps", tag="ps")
                        for dx in range(KX):
                            st, sp = dx == 0, dx == KX - 1
                            wc = slice(dx * 64, dx * 64 + 64)
                            xw = slice(dx, dx + XC)
                            nc.tensor.matmul(
                                out=psa[0:64, :],
                                lhsT=wsp_sb[0:NR, wc],
                                rhs=sl_v[0:NR, tv, zp, 0, xw, :],
                                start=st, stop=sp, tile_position=(0, 0),
                            )
                            nc.tensor.matmul(
                                out=psa[64:128, :],
                                lhsT=wsp_sb[64:64 + NR, wc],
                                rhs=sl_v[64:64 + NR, tv, zp, 1, xw, :],
                                start=st, stop=sp, tile_position=(64, 64),
                            )
                            nc.tensor.matmul(
                                out=psb[64:128, :],
                                lhsT=wsp_sb[0:NR, wc],
                                rhs=sl_v[0:NR, tv, zp, 1, xw, :],
                                start=st, stop=sp, tile_position=(0, 64),
                            )
                            nc.tensor.matmul(
                                out=psb[0:64, :],
                                lhsT=wsp_sb[64:64 + NR, wc],
                                rhs=sl_v[64:64 + NR, tv, zp, 0, xw, :],
                                start=st, stop=sp, tile_position=(64, 0),
                            )
                        # slices: slot0 = [(ze,j0) lo; (zo,j1) hi],
                        #         slot1 = [(zo,j0) lo; (ze,j1) hi]
                        a0 = zp * 1024
                        evac(slc[:, a0:a0 + 512], psa, (t + zp) % 2 == 0)
                        evac(slc[:, a0 + 512:a0 + 1024], psb, (t + zp) % 2 == 1)

                # ---- temporal phase ----
                # Col half = j (address slot), row half = z parity.
                def temporal(t):
                    taps = _temporal_taps(t)
                    tmp = tmp_pool.tile([128, ZB * 512], BF16, name="tmp", tag="tmp")
                    for zp in range(ZB // 2):
                        psa = psum_pool.tile([128, 512], F32, name="ps", tag="ps")
                        psb = psum_pool.tile([128, 512], F32, name="ps", tag="ps")
                        a0 = zp * 1024
                        for a, (s, v) in enumerate(taps):
                            st = a == 0
                            sp = a == len(taps) - 1
                            vsl = slices[s]
                            c0, c1 = v * 64, (v + 1) * 64
                            # bank A (free 0-511) = [(ze,j0); (ze,j1)],
                            # bank B = [(zo,j1) lo; (zo,j0) hi] (j-swapped;
                            # host undoes it for odd z)
                            nc.tensor.matmul(
                                out=psa[0:64, :],
                                lhsT=wtp_sb[0:64, c0:c1],
                                rhs=vsl[0:64, a0:a0 + 512],
                                start=st, stop=sp, tile_position=(0, 0),
                            )
                            nc.tensor.matmul(
                                out=psa[64:128, :],
                                lhsT=wtp_sb[64:128, c0:c1],
                                rhs=vsl[64:128, a0 + 512:a0 + 1024],
                                start=st, stop=sp, tile_position=(64, 64),
                            )
                            nc.tensor.matmul(
                                out=psb[64:128, :],
                                lhsT=wtp_sb[0:64, c0:c1],
                                rhs=vsl[0:64, a0 + 512:a0 + 1024],
                                start=st, stop=sp, tile_position=(0, 64),
                            )
                            nc.tensor.matmul(
                                out=psb[0:64, :],
                                lhsT=wtp_sb[64:128, c0:c1],
                                rhs=vsl[64:128, a0:a0 + 512],
                                start=st, stop=sp, tile_position=(64, 0),
                            )
                        evac(tmp[:, a0:a0 + 512], psa, (t + zp) % 2 == 1)
                        evac(tmp[:, a0 + 512:a0 + 1024], psb, (t + zp) % 2 == 0)
                    # one ~0.5MB DMA per (t, z-block): 4KB-aligned runs;
                    # alternate rings so both HWDGE rings carry ~17MB
                    eng = nc.sync if t % 2 == 0 else nc.scalar
                    eng.dma_start(
                        out=outq[t, :, z0:z0 + ZB, :],
                        in_=tmp.rearrange("p (z xy) -> p z xy", z=ZB),
                    )

                # interleave: temporal(t) after spatial(t+2), so output
                # DMAs and evacuations spread evenly across the z-block
                # AND temporal's last tap never waits on a slice evac
                # that finished only ~1us earlier
                spatial(0)
                if zb + 1 < NZB:
                    next_slab = load_slab(zb + 1)
                spatial(1)
                for t in range(2, T):
                    spatial(t)
                    temporal(t - 2)
                temporal(T - 2)
                temporal(T - 1)

    nc.finalize()
    return nc


def _prep_inputs(xr, xi, wxyz_r, wxyz_i, wt_r, wt_i):
    xr = np.asarray(xr, np.float32)
    xi = np.asarray(xi, np.float32)

    wsr, wsi = _project(np.asarray(wxyz_r, np.float64), np.asarray(wxyz_i, np.float64), True)
    wtr, wti = _project(np.asarray(wt_r, np.float64), np.asarray(wt_i, np.float64), False)
    wsp = _spatial_lhsT(wsr, wsi)
    wtp = _temporal_lhsT(wtr, wti)

    pads = [(0, 0), (0, 0), (1, 1), (1, 1), (1, 1), (0, 0)]
    xp = np.stack([np.pad(xr, pads, mode="symmetric"),
                   np.pad(xi, pads, mode="symmetric")])  # [ri2, B, T, ZP, YP, XP, C]
    xp = xp.astype(BF16NP)
    in_maps = []
    for core in range(8):
        b, cx = divmod(core, NXC)
        xs = xp[:, b, :, :, :, XC * cx:XC * cx + XI, :]   # [ri2, T, ZP, YP, XI, C]
        xin = np.empty((NR, T, Z, RW), BF16NP)
        for dz in range(KZ):
            for dy in range(KY):
                blk = xs[:, :, dz:dz + Z, dy:dy + Y, :, :]     # [ri,T,Z,Y,XI,C]
                blk = blk.reshape(2, T, Z, 2, 32, XI, C)       # y -> (j, y')
                blk = blk.transpose(6, 0, 1, 2, 3, 5, 4)       # [C,ri,T,Z,j,XI,y']
                blk = blk.reshape(4, T, Z, RW)
                r0 = ((dz * 3 + dy) * 4)
                xin[r0:r0 + 4] = blk
        # [r, zb, t, zp, row] zb-major, split into 2048-elem pieces each
        # stored in its own 4096-elem (two-page) slot, data page-aligned
        def pack(par):                                  # par: 0=even z, 1=odd
            v = xin[:, :, par::2].reshape(NR, T, NZB, 2, RW)
            v = v.transpose(0, 2, 1, 3, 4).reshape(NR, NZB, NPC, 2048)
            out = np.zeros((NR, NZB, NPC, 4096), BF16NP)
            out[:, :, :, 0:2048] = v
            return out

        def pack0(par):                                 # zb0 t-pair chunks
            v = xin[:, :, par:ZB:2].reshape(NR, 4, 2 * 2304)
            return np.ascontiguousarray(v)
        in_maps.append({"xev": pack(0), "xod": pack(1),
                        "xev0": pack0(0), "xod0": pack0(1),
                        "wsp": wsp, "wtp": wtp})
    return in_maps


def kernel(xr, xi, wxyz_r, wxyz_i, wt_r, wt_i):
    if "nc" not in _NC_CACHE:
        _NC_CACHE["nc"] = build_program()
    nc = _NC_CACHE["nc"]

    in_maps = _prep_inputs(xr, xi, wxyz_r, wxyz_i, wt_r, wt_i)
    res = run_bass_kernel_spmd(nc, in_maps, list(range(8)))

    yr = np.empty((B, T, Z, Y, X, F), np.float32)
    yi = np.empty((B, T, Z, Y, X, F), np.float32)
    for core in range(8):
        b, cx = divmod(core, NXC)
        # outq[t, 64j+32q'+f, z, 32x+y'] -> y[t, z, 32j+y', x, f];
        # odd z rows store j swapped
        arr = np.asarray(res.results[core]["outq"], dtype=BF16NP).astype(np.float32)
        arr = arr.transpose(0, 2, 1, 3)               # [t,z,128,512]
        arr = arr.reshape(T, Z, 2, 2, F, XC, 32)      # [t,z,j,q',f,x,y']
        arr[:, 1::2] = arr[:, 1::2, ::-1]
        arr = arr.transpose(0, 1, 2, 6, 5, 4, 3)      # [t,z,j,y',x,f,q']
        arr = arr.reshape(T, Z, Y, XC, F, 2)
        yr[b, :, :, :, XC * cx:XC * cx + XC, :] = arr[..., 0]
        yi[b, :, :, :, XC * cx:XC * cx + XC, :] = arr[..., 1]
    return yr, yi


# revision 48
# speedup vs baseline: 1.2171x; 1.2171x over previous
"""Complex 3D+temporal conv (ComplexPadConv3Dt) on 8 Trainium2 NeuronCores.

Strategy (hardcoded for B=2, T=8, Z=20, Y=64, X=64, C=2, F1=F=32, k=3):
 - Pure data-parallel sharding: 8 cores = B(2) x X-quarters(4). Each core
   computes its (b, 16-wide x slab) including halo; no collectives.
 - All matmuls bf16 (rel err ~5e-3 vs the 2e-2 gate), PSUM accumulates f32.
 - The PE overlaps a 4-matmul quadrant wave fully (~216ns, the N=512
   streaming time) only when the two tiles in each column-half stream the
   SAME rhs address into both partition halves. Both phases are built
   around such waves:
   * Spatial conv: K=36 contraction (dz,dy)x(c,ri), dz/dy baked into the
     DRAM relayout, dx as a free-dim x offset (3 accumulating waves).
     SBUF slab partitions 0-35 hold even-z (z,j)-addressed data;
     partitions 64-99 hold the odd-z data at the same addresses, so one
     (zp,j) address feeds z=even from the low row half and z=odd from
     the high row half. Only even-z addresses are ever streamed, so the
     main load ships just the even-z rows (halves main input DMA).
   * Per (t, z-pair) outputs land in a [128,1024] 2-bank PSUM tile; the
     bf16 slices copy is slot0 = [(ze,j0) lo; (zo,j1) hi], slot1 =
     [(zo,j0) lo; (ze,j1) hi].
   * Temporal conv: K=64 contraction (q,f1), 3 taps accumulated; col
     half = j address slot, row half = z parity; bank ze comes out
     straight [(ze,j0); (ze,j1)], bank zo j-swapped (host undoes it).
 - Evacuations split each 2-bank PSUM tile across BOTH psum-capable
   engines (ScalarE bank A, DVE bank B, alternating) so the tile frees
   in ~0.66us and the 4-deep psum rotation keeps the PE fed.
 - DMA layout tuned for the SDMA engines' ~55ns/packet overhead and
   DRAM-page splits: every stream moves in 4KB-aligned pieces.
   * Inputs: per (row, z-block) the whole 8-t run (36864B) is contiguous
     in DRAM, padded to a 40960B slot -> nine perfect 4KB packets. One
     main DMA (rows 0-35, even SDMA engines, ScalarE ring) and one
     swapped-copy DMA (rows 64-99, odd engines, GpSimd SWDGE ring) per
     z-block: the two halves load in parallel on disjoint engines.
   * Temporal results accumulate per (t, z-block) into a [128, 2048]
     bf16 tile, DMA'd as ONE ~0.5MB transfer with 4KB-aligned
     per-partition runs into outq[T, 128, Z, 512] (host un-permutes).
 - temporal(t) issues after spatial(t+2): output DMAs/evacs spread
   evenly and temporal's last tap never chases a just-written slice.
 - ~3.5us of dummy matmuls at startup open the PE HAM clock gate to
   2.4GHz before the first real wave.
 - Outputs stored bf16, upcast on host.
"""

import numpy as np
import ml_dtypes

import concourse.bass as bass
import concourse.bacc as bacc
import concourse.mybir as mybir
from concourse import tile
from concourse.bass_utils import run_bass_kernel_spmd

# Problem constants
B, T, Z, Y, X, C = 2, 8, 20, 64, 64, 2
F1, F = 32, 32
KZ = KY = KX = 3
KT = 3

# Sharding / tiling
XC = 16          # output x columns per core
NXC = X // XC    # 4 x-chunks
XI = XC + 2      # input x columns per core (halo)
ZB = 4           # z rows per block
NZB = Z // ZB    # 5 blocks
NR = 36          # spatial contraction rows (dz,dy,c,ri)
RW = 2 * XI * 32          # 1152 elems per (z, t) row
ZROW = T * 2 * RW         # 18432 elems per (r, zb): 8t x 2zp x 1152
NPC = 9                   # ZROW = nine 2048-elem (4KB = one DRAM page) pieces
# each piece sits in a 4096-elem (8KB) slot: the non-contiguous stride
# stops bass's AP optimizer from re-merging pieces into one big run, so
# every DMA descriptor is exactly one aligned DRAM page (25.8GB/s/engine
# measured, vs 13.3 for a merged 36KB run that crosses 8 page boundaries)

F32 = mybir.dt.float32
BF16 = mybir.dt.bfloat16
BF16NP = ml_dtypes.bfloat16

_NC_CACHE = {}


def _project(wr, wi, zero_mean):
    wr = wr.astype(np.float64)
    wi = wi.astype(np.float64)
    ax = (0, 1, 2, 3)
    if zero_mean:
        wr = wr - wr.mean(ax, keepdims=True)
        wi = wi - wi.mean(ax, keepdims=True)
    norm = np.sqrt((wr * wr + wi * wi).sum(ax, keepdims=True))
    s = 1.0 / np.maximum(norm, 1.0)
    return wr * s, wi * s


def _spatial_lhsT(wsr, wsi):
    """[128, 3*64] bf16. Col block dx; rows r = (dz*3+dy)*4 + c*2 + ri at
    partitions 0-35 and duplicated at 64-99. Cols: q'*32 + f."""
    w = np.zeros((128, 3 * 64), np.float64)
    for dx in range(KX):
        for dz in range(KZ):
            for dy in range(KY):
                for c in range(C):
                    r0 = (dz * 3 + dy) * 4 + c * 2
                    col = dx * 64
                    wr = wsr[dz, dy, dx, c, :]
                    wi = wsi[dz, dy, dx, c, :]
                    for base in (0, 64):
                        w[base + r0 + 0, col + 0:col + 32] = wr
                        w[base + r0 + 0, col + 32:col + 64] = wi
                        w[base + r0 + 1, col + 0:col + 32] = -wi
                        w[base + r0 + 1, col + 32:col + 64] = wr
    return w.astype(BF16NP)


def _temporal_lhsT(wtr, wti):
    """[128, 5*64] bf16. rows 64d + q*32 + f1 (q=0 spr, 1 spi); cols q'*32 + f.

    variants v: [wt0, wt1, wt2, wt0+wt1, wt1+wt2]
    """
    wtr = wtr.reshape(KT, F1, F)
    wti = wti.reshape(KT, F1, F)
    variants = [
        (wtr[0], wti[0]),
        (wtr[1], wti[1]),
        (wtr[2], wti[2]),
        (wtr[0] + wtr[1], wti[0] + wti[1]),
        (wtr[1] + wtr[2], wti[1] + wti[2]),
    ]
    w = np.zeros((64, 5 * 64), np.float64)
    for v, (vr, vi) in enumerate(variants):
        w[0:32, v * 64 + 0:v * 64 + 32] = vr          # spr -> yr
        w[0:32, v * 64 + 32:v * 64 + 64] = vi         # spr -> yi
        w[32:64, v * 64 + 0:v * 64 + 32] = -vi        # spi -> yr
        w[32:64, v * 64 + 32:v * 64 + 64] = vr        # spi -> yi
    out = np.zeros((128, 5 * 64), np.float64)
    out[0:64] = w
    out[64:128] = w
    return out.astype(BF16NP)


def _temporal_taps(t):
    if t == 0:
        return [(0, 3), (1, 2)]
    if t == T - 1:
        return [(T - 2, 0), (T - 1, 4)]
    return [(t - 1, 0), (t, 1), (t + 1, 2)]


def build_program():
    nc = bacc.Bacc(None, target_bir_lowering=False)

    # xev: even-z rows, xod: odd-z rows (the "z-swapped" copy). Per
    # (r, zb) one contiguous 36864B run in a 40960B (page-aligned) slot.
    xev = nc.declare_dram_parameter("xev", [NR, NZB, NPC, 4096], BF16, isOutput=False)
    xod = nc.declare_dram_parameter("xod", [NR, NZB, NPC, 4096], BF16, isOutput=False)
    # startup fast path: z-block 0 duplicated as four t-pair chunks so
    # the first waves launch after ~0.17MB instead of ~2.7MB of DMA
    xev0 = nc.declare_dram_parameter("xev0", [NR, 4, 2 * 2304], BF16, isOutput=False)
    xod0 = nc.declare_dram_parameter("xod0", [NR, 4, 2 * 2304], BF16, isOutput=False)
    wsp = nc.declare_dram_parameter("wsp", [128, 3 * 64], BF16, isOutput=False)
    wtp = nc.declare_dram_parameter("wtp", [128, 5 * 64], BF16, isOutput=False)
    outq = nc.declare_dram_parameter("outq", [T, 128, Z, 512], BF16, isOutput=True)

    with tile.TileContext(nc) as tc:
        with (
            tc.tile_pool(name="wpool", bufs=1) as wpool,
            tc.tile_pool(name="slabs", bufs=2) as slab_pool,
            tc.tile_pool(name="slab0", bufs=4) as slab0_pool,
            tc.tile_pool(name="slices", bufs=8) as slice_pool,
            tc.tile_pool(name="tmp", bufs=5) as tmp_pool,
            tc.tile_pool(name="psum", bufs=8, space="PSUM") as psum_pool,
        ):
            wsp_sb = wpool.tile([128, 3 * 64], BF16, name="wsp_sb", tag="wsp")
            wtp_sb = wpool.tile([128, 5 * 64], BF16, name="wtp_sb", tag="wtp")
            nc.sync.dma_start(out=wsp_sb[:], in_=wsp[:])
            nc.sync.dma_start(out=wtp_sb[:], in_=wtp[:])

            def load_slab(zb):
                # whole z-block slab [100, 8t x 2zp x 1152]; rows 0-35 =
                # even z, rows 64-99 = odd z at the same addresses. Two
                # DMAs on separate rings (ScalarE HWDGE / GpSimd SWDGE)
                # landing on disjoint (even/odd) SDMA engine sets.
                sl = slab_pool.tile([100, ZROW], BF16, name="sl", tag="sl")
                sl_k = sl.rearrange("p (k r) -> p k r", k=NPC)
                # both on the ScalarE HWDGE ring: its SDMA-engine set (12
                # observed) beats the GpSimd SWDGE ring's 4
                nc.scalar.dma_start(
                    out=sl_k[0:NR], in_=xev[:, zb, :, 0:2048]
                )
                nc.scalar.dma_start(
                    out=sl_k[64:64 + NR], in_=xod[:, zb, :, 0:2048]
                )
                return sl.rearrange(
                    "p (t z j x y) -> p t z j x y", t=T, z=2, j=2, x=XI, y=32
                )

            def load_slab0(tp):
                sl = slab0_pool.tile([100, 2 * 2304], BF16, name="sl0", tag="sl0")
                nc.scalar.dma_start(out=sl[0:NR, :], in_=xev0[:, tp])
                nc.scalar.dma_start(out=sl[64:64 + NR, :], in_=xod0[:, tp])
                return sl.rearrange(
                    "p (t z j x y) -> p t z j x y", t=2, z=2, j=2, x=XI, y=32
                )

            slab0 = [load_slab0(tp) for tp in range(4)]

            # HAM warmup: ~4-6us of dummy matmuls (on resident weight
            # tiles, into the first psum slot, overwritten later by the
            # first real start=True wave) so the PE clock-gate opens to
            # 2.4GHz while the first slab loads.
            warm_ps = psum_pool.tile([128, 512], F32, name="wps", tag="ps")
            for _ in range(64):
                nc.tensor.matmul(
                    out=warm_ps[0:64, 0:192],
                    lhsT=wsp_sb[:, 0:64], rhs=wsp_sb[:, 0:192],
                    start=False, stop=False, tile_position=(0, 0),
                )

            def evac(dst, psb, flip):
                # whole single-bank tile to one engine, alternating: the
                # 8-deep psum rotation gives the loop latency ~2.3us of
                # budget, so neither the PE nor the evac engines ever
                # block on a slot return
                if flip:
                    nc.scalar.copy(dst, psb[:, :])
                else:
                    nc.vector.tensor_copy(dst, psb[:, :])

            next_slab = None
            for zb in range(NZB):
                z0 = zb * ZB
                if zb == 0:
                    def rhs_view(t):
                        return slab0[t // 2], t % 2
                else:
                    sl_whole = next_slab

                    def rhs_view(t, _s=sl_whole):
                        return _s, t

                # ---- spatial phase ----
                # Per (t, z-pair): [128,1024]: bank A (free 0-511) =
                # [(ze,j0); (zo,j1)], bank B = [(zo,j0) lo; (ze,j1) hi].
                # Wave: col half = j address; row half lo = ze data, hi =
                # zo data (odd-z rows); same col half streams one address.
                slices = [None] * T

                def spatial(t):
                    slc = slice_pool.tile([128, ZB * 512], BF16, name="slc", tag="slc")
                    slices[t] = slc
                    sl_v, tv = rhs_view(t)
                    for zp in range(ZB // 2):
                        psa = psum_pool.tile([128, 512], F32, name="ps", tag="ps")
                        psb = psum_pool.tile([128, 512], F32, name="ps", tag="ps")
                        for dx in range(KX):
                            st, sp = dx == 0, dx == KX - 1
                            wc = slice(dx * 64, dx * 64 + 64)
                            xw = slice(dx, dx + XC)
                            nc.tensor.matmul(
                                out=psa[0:64, :],
                                lhsT=wsp_sb[0:NR, wc],
                                rhs=sl_v[0:NR, tv, zp, 0, xw, :],
                                start=st, stop=sp, tile_position=(0, 0),
                            )
                            nc.tensor.matmul(
                                out=psa[64:128, :],
                                lhsT=wsp_sb[64:64 + NR, wc],
                                rhs=sl_v[64:64 + NR, tv, zp, 1, xw, :],
                                start=st, stop=sp, tile_position=(64, 64),
                            )
                            nc.tensor.matmul(
                                out=psb[64:128, :],
                                lhsT=wsp_sb[0:NR, wc],
                                rhs=sl_v[0:NR, tv, zp, 1, xw, :],
                                start=st, stop=sp, tile_position=(0, 64),
                            )
                            nc.tensor.matmul(
                                out=psb[0:64, :],
                                lhsT=wsp_sb[64:64 + NR, wc],
                                rhs=sl_v[64:64 + NR, tv, zp, 0, xw, :],
                                start=st, stop=sp, tile_position=(64, 0),
                            )
                        # slices: slot0 = [(ze,j0) lo; (zo,j1) hi],
                        #         slot1 = [(zo,j0) lo; (ze,j1) hi]
                        a0 = zp * 1024
                        evac(slc[:, a0:a0 + 512], psa, (t + zp) % 2 == 0)
                        evac(slc[:, a0 + 512:a0 + 1024], psb, (t + zp) % 2 == 1)

                # ---- temporal phase ----
                # Col half = j (address slot), row half = z parity.
                def temporal(t):
                    taps = _temporal_taps(t)
                    tmp = tmp_pool.tile([128, ZB * 512], BF16, name="tmp", tag="tmp")
                    for zp in range(ZB // 2):
                        psa = psum_pool.tile([128, 512], F32, name="ps", tag="ps")
                        psb = psum_pool.tile([128, 512], F32, name="ps", tag="ps")
                        a0 = zp * 1024
                        for a, (s, v) in enumerate(taps):
                            st = a == 0
                            sp = a == len(taps) - 1
                            vsl = slices[s]
                            c0, c1 = v * 64, (v + 1) * 64
                            # bank A (free 0-511) = [(ze,j0); (ze,j1)],
                            # bank B = [(zo,j1) lo; (zo,j0) hi] (j-swapped;
                            # host undoes it for odd z)
                            nc.tensor.matmul(
                                out=psa[0:64, :],
                                lhsT=wtp_sb[0:64, c0:c1],
                                rhs=vsl[0:64, a0:a0 + 512],
                                start=st, stop=sp, tile_position=(0, 0),
                            )
                            nc.tensor.matmul(
                                out=psa[64:128, :],
                                lhsT=wtp_sb[64:128, c0:c1],
                                rhs=vsl[64:128, a0 + 512:a0 + 1024],
                                start=st, stop=sp, tile_position=(64, 64),
                            )
                            nc.tensor.matmul(
                                out=psb[64:128, :],
                                lhsT=wtp_sb[0:64, c0:c1],
                                rhs=vsl[0:64, a0 + 512:a0 + 1024],
                                start=st, stop=sp, tile_position=(0, 64),
                            )
                            nc.tensor.matmul(
                                out=psb[0:64, :],
                                lhsT=wtp_sb[64:128, c0:c1],
                                rhs=vsl[64:128, a0:a0 + 512],
                                start=st, stop=sp, tile_position=(64, 0),
                            )
                        evac(tmp[:, a0:a0 + 512], psa, (t + zp) % 2 == 1)
                        evac(tmp[:, a0 + 512:a0 + 1024], psb, (t + zp) % 2 == 0)
                    # one ~0.5MB DMA per (t, z-block): 4KB-aligned runs
                    nc.sync.dma_start(
                        out=outq[t, :, z0:z0 + ZB, :],
                        in_=tmp.rearrange("p (z xy) -> p z xy", z=ZB),
                    )

                # interleave: temporal(t) after spatial(t+2), so output
                # DMAs and evacuations spread evenly across the z-block
                # AND temporal's last tap never waits on a slice evac
                # that finished only ~1us earlier
                spatial(0)
                if zb + 1 < NZB:
                    next_slab = load_slab(zb + 1)
                spatial(1)
                for t in range(2, T):
                    spatial(t)
                    temporal(t - 2)
                temporal(T - 2)
                temporal(T - 1)

    nc.finalize()
    return nc


def _prep_inputs(xr, xi, wxyz_r, wxyz_i, wt_r, wt_i):
    xr = np.asarray(xr, np.float32)
    xi = np.asarray(xi, np.float32)

    wsr, wsi = _project(np.asarray(wxyz_r, np.float64), np.asarray(wxyz_i, np.float64), True)
    wtr, wti = _project(np.asarray(wt_r, np.float64), np.asarray(wt_i, np.float64), False)
    wsp = _spatial_lhsT(wsr, wsi)
    wtp = _temporal_lhsT(wtr, wti)

    pads = [(0, 0), (0, 0), (1, 1), (1, 1), (1, 1), (0, 0)]
    xp = np.stack([np.pad(xr, pads, mode="symmetric"),
                   np.pad(xi, pads, mode="symmetric")])  # [ri2, B, T, ZP, YP, XP, C]
    xp = xp.astype(BF16NP)
    in_maps = []
    for core in range(8):
        b, cx = divmod(core, NXC)
        xs = xp[:, b, :, :, :, XC * cx:XC * cx + XI, :]   # [ri2, T, ZP, YP, XI, C]
        xin = np.empty((NR, T, Z, RW), BF16NP)
        for dz in range(KZ):
            for dy in range(KY):
                blk = xs[:, :, dz:dz + Z, dy:dy + Y, :, :]     # [ri,T,Z,Y,XI,C]
                blk = blk.reshape(2, T, Z, 2, 32, XI, C)       # y -> (j, y')
                blk = blk.transpose(6, 0, 1, 2, 3, 5, 4)       # [C,ri,T,Z,j,XI,y']
                blk = blk.reshape(4, T, Z, RW)
                r0 = ((dz * 3 + dy) * 4)
                xin[r0:r0 + 4] = blk
        # [r, zb, t, zp, row] zb-major, split into 2048-elem pieces each
        # stored in its own 4096-elem (two-page) slot, data page-aligned
        def pack(par):                                  # par: 0=even z, 1=odd
            v = xin[:, :, par::2].reshape(NR, T, NZB, 2, RW)
            v = v.transpose(0, 2, 1, 3, 4).reshape(NR, NZB, NPC, 2048)
            out = np.zeros((NR, NZB, NPC, 4096), BF16NP)
            out[:, :, :, 0:2048] = v
            return out

        def pack0(par):                                 # zb0 t-pair chunks
            v = xin[:, :, par:ZB:2].reshape(NR, 4, 2 * 2304)
            return np.ascontiguousarray(v)
        in_maps.append({"xev": pack(0), "xod": pack(1),
                        "xev0": pack0(0), "xod0": pack0(1),
                        "wsp": wsp, "wtp": wtp})
    return in_maps


def kernel(xr, xi, wxyz_r, wxyz_i, wt_r, wt_i):
    if "nc" not in _NC_CACHE:
        _NC_CACHE["nc"] = build_program()
    nc = _NC_CACHE["nc"]

    in_maps = _prep_inputs(xr, xi, wxyz_r, wxyz_i, wt_r, wt_i)
    res = run_bass_kernel_spmd(nc, in_maps, list(range(8)))

    yr = np.empty((B, T, Z, Y, X, F), np.float32)
    yi = np.empty((B, T, Z, Y, X, F), np.float32)
    for core in range(8):
        b, cx = divmod(core, NXC)
        # outq[t, 64j+32q'+f, z, 32x+y'] -> y[t, z, 32j+y', x, f];
        # odd z rows store j swapped
        arr = np.asarray(res.results[core]["outq"], dtype=BF16NP).astype(np.float32)
        arr = arr.transpose(0, 2, 1, 3)               # [t,z,128,512]
        arr = arr.reshape(T, Z, 2, 2, F, XC, 32)      # [t,z,j,q',f,x,y']
        arr[:, 1::2] = arr[:, 1::2, ::-1]
        arr = arr.transpose(0, 1, 2, 6, 5, 4, 3)      # [t,z,j,y',x,f,q']
        arr = arr.reshape(T, Z, Y, XC, F, 2)
        yr[b, :, :, :, XC * cx:XC * cx + XC, :] = arr[..., 0]
        yi[b, :, :, :, XC * cx:XC * cx + XC, :] = arr[..., 1]
    return yr, yi


# revision 49
# speedup vs baseline: 1.2304x; 1.0109x over previous
"""Complex 3D+temporal conv (ComplexPadConv3Dt) on 8 Trainium2 NeuronCores.

Strategy (hardcoded for B=2, T=8, Z=20, Y=64, X=64, C=2, F1=F=32, k=3):
 - Pure data-parallel sharding: 8 cores = B(2) x X-quarters(4). Each core
   computes its (b, 16-wide x slab) including halo; no collectives.
 - All matmuls bf16 (rel err ~5e-3 vs the 2e-2 gate), PSUM accumulates f32.
 - The PE overlaps a 4-matmul quadrant wave fully (~216ns, the N=512
   streaming time) only when the two tiles in each column-half stream the
   SAME rhs address into both partition halves. Both phases are built
   around such waves:
   * Spatial conv: K=36 contraction (dz,dy)x(c,ri), dz/dy baked into the
     DRAM relayout, dx as a free-dim x offset (3 accumulating waves).
     SBUF slab partitions 0-35 hold even-z (z,j)-addressed data;
     partitions 64-99 hold the odd-z data at the same addresses, so one
     (zp,j) address feeds z=even from the low row half and z=odd from
     the high row half. Only even-z addresses are ever streamed, so the
     main load ships just the even-z rows (halves main input DMA).
   * Per (t, z-pair) outputs land in a [128,1024] 2-bank PSUM tile; the
     bf16 slices copy is slot0 = [(ze,j0) lo; (zo,j1) hi], slot1 =
     [(zo,j0) lo; (ze,j1) hi].
   * Temporal conv: K=64 contraction (q,f1), 3 taps accumulated; col
     half = j address slot, row half = z parity; bank ze comes out
     straight [(ze,j0); (ze,j1)], bank zo j-swapped (host undoes it).
 - Evacuations split each 2-bank PSUM tile across BOTH psum-capable
   engines (ScalarE bank A, DVE bank B, alternating) so the tile frees
   in ~0.66us and the 4-deep psum rotation keeps the PE fed.
 - DMA layout tuned for the SDMA engines' ~55ns/packet overhead and
   DRAM-page splits: every stream moves in 4KB-aligned pieces.
   * Inputs: per (row, z-block) the whole 8-t run (36864B) is contiguous
     in DRAM, padded to a 40960B slot -> nine perfect 4KB packets. One
     main DMA (rows 0-35, even SDMA engines, ScalarE ring) and one
     swapped-copy DMA (rows 64-99, odd engines, GpSimd SWDGE ring) per
     z-block: the two halves load in parallel on disjoint engines.
   * Temporal results accumulate per (t, z-block) into a [128, 2048]
     bf16 tile, DMA'd as ONE ~0.5MB transfer with 4KB-aligned
     per-partition runs into outq[T, 128, Z, 512] (host un-permutes).
 - temporal(t) issues after spatial(t+2): output DMAs/evacs spread
   evenly and temporal's last tap never chases a just-written slice.
 - ~3.5us of dummy matmuls at startup open the PE HAM clock gate to
   2.4GHz before the first real wave.
 - Outputs stored bf16, upcast on host.
"""

import numpy as np
import ml_dtypes

import concourse.bass as bass
import concourse.bacc as bacc
import concourse.mybir as mybir
from concourse import tile
from concourse.bass_utils import run_bass_kernel_spmd

# Problem constants
B, T, Z, Y, X, C = 2, 8, 20, 64, 64, 2
F1, F = 32, 32
KZ = KY = KX = 3
KT = 3

# Sharding / tiling
XC = 16          # output x columns per core
NXC = X // XC    # 4 x-chunks
XI = XC + 2      # input x columns per core (halo)
ZB = 4           # z rows per block
NZB = Z // ZB    # 5 blocks
NR = 36          # spatial contraction rows (dz,dy,c,ri)
RW = 2 * XI * 32          # 1152 elems per (z, t) row
ZROW = T * 2 * RW         # 18432 elems per (r, zb): 8t x 2zp x 1152
NPC = 9                   # ZROW = nine 2048-elem (4KB = one DRAM page) pieces
# each piece sits in a 4096-elem (8KB) slot: the non-contiguous stride
# stops bass's AP optimizer from re-merging pieces into one big run, so
# every DMA descriptor is exactly one aligned DRAM page (25.8GB/s/engine
# measured, vs 13.3 for a merged 36KB run that crosses 8 page boundaries)

F32 = mybir.dt.float32
BF16 = mybir.dt.bfloat16
BF16NP = ml_dtypes.bfloat16

_NC_CACHE = {}


def _project(wr, wi, zero_mean):
    wr = wr.astype(np.float64)
    wi = wi.astype(np.float64)
    ax = (0, 1, 2, 3)
    if zero_mean:
        wr = wr - wr.mean(ax, keepdims=True)
        wi = wi - wi.mean(ax, keepdims=True)
    norm = np.sqrt((wr * wr + wi * wi).sum(ax, keepdims=True))
    s = 1.0 / np.maximum(norm, 1.0)
    return wr * s, wi * s


def _spatial_lhsT(wsr, wsi):
    """[128, 3*64] bf16. Col block dx; rows r = (dz*3+dy)*4 + c*2 + ri at
    partitions 0-35 and duplicated at 64-99. Cols: q'*32 + f."""
    w = np.zeros((128, 3 * 64), np.float64)
    for dx in range(KX):
        for dz in range(KZ):
            for dy in range(KY):
                for c in range(C):
                    r0 = (dz * 3 + dy) * 4 + c * 2
                    col = dx * 64
                    wr = wsr[dz, dy, dx, c, :]
                    wi = wsi[dz, dy, dx, c, :]
                    for base in (0, 64):
                        w[base + r0 + 0, col + 0:col + 32] = wr
                        w[base + r0 + 0, col + 32:col + 64] = wi
                        w[base + r0 + 1, col + 0:col + 32] = -wi
                        w[base + r0 + 1, col + 32:col + 64] = wr
    return w.astype(BF16NP)


def _temporal_lhsT(wtr, wti):
    """[128, 5*64] bf16. rows 64d + q*32 + f1 (q=0 spr, 1 spi); cols q'*32 + f.

    variants v: [wt0, wt1, wt2, wt0+wt1, wt1+wt2]
    """
    wtr = wtr.reshape(KT, F1, F)
    wti = wti.reshape(KT, F1, F)
    variants = [
        (wtr[0], wti[0]),
        (wtr[1], wti[1]),
        (wtr[2], wti[2]),
        (wtr[0] + wtr[1], wti[0] + wti[1]),
        (wtr[1] + wtr[2], wti[1] + wti[2]),
    ]
    w = np.zeros((64, 5 * 64), np.float64)
    for v, (vr, vi) in enumerate(variants):
        w[0:32, v * 64 + 0:v * 64 + 32] = vr          # spr -> yr
        w[0:32, v * 64 + 32:v * 64 + 64] = vi         # spr -> yi
        w[32:64, v * 64 + 0:v * 64 + 32] = -vi        # spi -> yr
        w[32:64, v * 64 + 32:v * 64 + 64] = vr        # spi -> yi
    out = np.zeros((128, 5 * 64), np.float64)
    out[0:64] = w
    out[64:128] = w
    return out.astype(BF16NP)


def _temporal_taps(t):
    if t == 0:
        return [(0, 3), (1, 2)]
    if t == T - 1:
        return [(T - 2, 0), (T - 1, 4)]
    return [(t - 1, 0), (t, 1), (t + 1, 2)]


def build_program():
    nc = bacc.Bacc(None, target_bir_lowering=False)

    # xev: even-z rows, xod: odd-z rows (the "z-swapped" copy). Per
    # (r, zb) one contiguous 36864B run in a 40960B (page-aligned) slot.
    xev = nc.declare_dram_parameter("xev", [NR, NZB, NPC, 4096], BF16, isOutput=False)
    xod = nc.declare_dram_parameter("xod", [NR, NZB, NPC, 4096], BF16, isOutput=False)
    # startup fast path: z-block 0 duplicated as four t-pair chunks so
    # the first waves launch after ~0.17MB instead of ~2.7MB of DMA
    xev0 = nc.declare_dram_parameter("xev0", [NR, 4, 2 * 2304], BF16, isOutput=False)
    xod0 = nc.declare_dram_parameter("xod0", [NR, 4, 2 * 2304], BF16, isOutput=False)
    wsp = nc.declare_dram_parameter("wsp", [128, 3 * 64], BF16, isOutput=False)
    wtp = nc.declare_dram_parameter("wtp", [128, 5 * 64], BF16, isOutput=False)
    outq = nc.declare_dram_parameter("outq", [T, 128, Z, 512], BF16, isOutput=True)

    with tile.TileContext(nc) as tc:
        with (
            tc.tile_pool(name="wpool", bufs=1) as wpool,
            tc.tile_pool(name="slabs", bufs=2) as slab_pool,
            tc.tile_pool(name="slab0", bufs=4) as slab0_pool,
            tc.tile_pool(name="slices", bufs=9) as slice_pool,
            tc.tile_pool(name="tmp", bufs=6) as tmp_pool,
            tc.tile_pool(name="psum", bufs=8, space="PSUM") as psum_pool,
        ):
            wsp_sb = wpool.tile([128, 3 * 64], BF16, name="wsp_sb", tag="wsp")
            wtp_sb = wpool.tile([128, 5 * 64], BF16, name="wtp_sb", tag="wtp")
            nc.sync.dma_start(out=wsp_sb[:], in_=wsp[:])
            nc.sync.dma_start(out=wtp_sb[:], in_=wtp[:])

            def load_slab(zb):
                # whole z-block slab [100, 8t x 2zp x 1152]; rows 0-35 =
                # even z, rows 64-99 = odd z at the same addresses. Two
                # DMAs on separate rings (ScalarE HWDGE / GpSimd SWDGE)
                # landing on disjoint (even/odd) SDMA engine sets.
                sl = slab_pool.tile([100, ZROW], BF16, name="sl", tag="sl")
                sl_k = sl.rearrange("p (k r) -> p k r", k=NPC)
                # both on the ScalarE HWDGE ring: its SDMA-engine set (12
                # observed) beats the GpSimd SWDGE ring's 4
                nc.scalar.dma_start(
                    out=sl_k[0:NR], in_=xev[:, zb, :, 0:2048]
                )
                nc.scalar.dma_start(
                    out=sl_k[64:64 + NR], in_=xod[:, zb, :, 0:2048]
                )
                return sl.rearrange(
                    "p (t z j x y) -> p t z j x y", t=T, z=2, j=2, x=XI, y=32
                )

            def load_slab0(tp):
                sl = slab0_pool.tile([100, 2 * 2304], BF16, name="sl0", tag="sl0")
                nc.scalar.dma_start(out=sl[0:NR, :], in_=xev0[:, tp])
                nc.scalar.dma_start(out=sl[64:64 + NR, :], in_=xod0[:, tp])
                return sl.rearrange(
                    "p (t z j x y) -> p t z j x y", t=2, z=2, j=2, x=XI, y=32
                )

            slab0 = [load_slab0(tp) for tp in range(4)]

            # HAM warmup: ~4-6us of dummy matmuls (on resident weight
            # tiles, into the first psum slot, overwritten later by the
            # first real start=True wave) so the PE clock-gate opens to
            # 2.4GHz while the first slab loads.
            warm_ps = psum_pool.tile([128, 512], F32, name="wps", tag="ps")
            for _ in range(84):
                nc.tensor.matmul(
                    out=warm_ps[0:64, 0:192],
                    lhsT=wsp_sb[:, 0:64], rhs=wsp_sb[:, 0:192],
                    start=False, stop=False, tile_position=(0, 0),
                )

            def evac(dst, psb, flip):
                # whole single-bank tile to one engine, alternating: the
                # 8-deep psum rotation gives the loop latency ~2.3us of
                # budget, so neither the PE nor the evac engines ever
                # block on a slot return
                if flip:
                    nc.scalar.copy(dst, psb[:, :])
                else:
                    nc.vector.tensor_copy(dst, psb[:, :])

            next_slab = None
            for zb in range(NZB):
                z0 = zb * ZB
                if zb == 0:
                    def rhs_view(t):
                        return slab0[t // 2], t % 2
                else:
                    sl_whole = next_slab

                    def rhs_view(t, _s=sl_whole):
                        return _s, t

                # ---- spatial phase ----
                # Per (t, z-pair): [128,1024]: bank A (free 0-511) =
                # [(ze,j0); (zo,j1)], bank B = [(zo,j0) lo; (ze,j1) hi].
                # Wave: col half = j address; row half lo = ze data, hi =
                # zo data (odd-z rows); same col half streams one address.
                slices = [None] * T

                def spatial(t):
                    slc = slice_pool.tile([128, ZB * 512], BF16, name="slc", tag="slc")
                    slices[t] = slc
                    sl_v, tv = rhs_view(t)
                    for zp in range(ZB // 2):
                        psa = psum_pool.tile([128, 512], F32, name="ps", tag="ps")
                        psb = psum_pool.tile([128, 512], F32, name="ps", tag="ps")
                        for dx in range(KX):
                            st, sp = dx == 0, dx == KX - 1
                            wc = slice(dx * 64, dx * 64 + 64)
                            xw = slice(dx, dx + XC)
                            nc.tensor.matmul(
                                out=psa[0:64, :],
                                lhsT=wsp_sb[0:NR, wc],
                                rhs=sl_v[0:NR, tv, zp, 0, xw, :],
                                start=st, stop=sp, tile_position=(0, 0),
                            )
                            nc.tensor.matmul(
                                out=psa[64:128, :],
                                lhsT=wsp_sb[64:64 + NR, wc],
                                rhs=sl_v[64:64 + NR, tv, zp, 1, xw, :],
                                start=st, stop=sp, tile_position=(64, 64),
                            )
                            nc.tensor.matmul(
                                out=psb[64:128, :],
                                lhsT=wsp_sb[0:NR, wc],
                                rhs=sl_v[0:NR, tv, zp, 1, xw, :],
                                start=st, stop=sp, tile_position=(0, 64),
                            )
                            nc.tensor.matmul(
                                out=psb[0:64, :],
                                lhsT=wsp_sb[64:64 + NR, wc],
                                rhs=sl_v[64:64 + NR, tv, zp, 0, xw, :],
                                start=st, stop=sp, tile_position=(64, 0),
                            )
                        # slices: slot0 = [(ze,j0) lo; (zo,j1) hi],
                        #         slot1 = [(zo,j0) lo; (ze,j1) hi]
                        a0 = zp * 1024
                        evac(slc[:, a0:a0 + 512], psa, (t + zp) % 2 == 0)
                        evac(slc[:, a0 + 512:a0 + 1024], psb, (t + zp) % 2 == 1)

                # ---- temporal phase ----
                # Col half = j (address slot), row half = z parity.
                def temporal(t):
                    taps = _temporal_taps(t)
                    tmp = tmp_pool.tile([128, ZB * 512], BF16, name="tmp", tag="tmp")
                    for zp in range(ZB // 2):
                        psa = psum_pool.tile([128, 512], F32, name="ps", tag="ps")
                        psb = psum_pool.tile([128, 512], F32, name="ps", tag="ps")
                        a0 = zp * 1024
                        for a, (s, v) in enumerate(taps):
                            st = a == 0
                            sp = a == len(taps) - 1
                            vsl = slices[s]
                            c0, c1 = v * 64, (v + 1) * 64
                            # bank A (free 0-511) = [(ze,j0); (ze,j1)],
                            # bank B = [(zo,j1) lo; (zo,j0) hi] (j-swapped;
                            # host undoes it for odd z)
                            nc.tensor.matmul(
                                out=psa[0:64, :],
                                lhsT=wtp_sb[0:64, c0:c1],
                                rhs=vsl[0:64, a0:a0 + 512],
                                start=st, stop=sp, tile_position=(0, 0),
                            )
                            nc.tensor.matmul(
                                out=psa[64:128, :],
                                lhsT=wtp_sb[64:128, c0:c1],
                                rhs=vsl[64:128, a0 + 512:a0 + 1024],
                                start=st, stop=sp, tile_position=(64, 64),
                            )
                            nc.tensor.matmul(
                                out=psb[64:128, :],
                                lhsT=wtp_sb[0:64, c0:c1],
                                rhs=vsl[0:64, a0 + 512:a0 + 1024],
                                start=st, stop=sp, tile_position=(0, 64),
                            )
                            nc.tensor.matmul(
                                out=psb[0:64, :],
                                lhsT=wtp_sb[64:128, c0:c1],
                                rhs=vsl[64:128, a0:a0 + 512],
                                start=st, stop=sp, tile_position=(64, 0),
                            )
                        evac(tmp[:, a0:a0 + 512], psa, (t + zp) % 2 == 1)
                        evac(tmp[:, a0 + 512:a0 + 1024], psb, (t + zp) % 2 == 0)
                    # one ~0.5MB DMA per (t, z-block): 4KB-aligned runs
                    nc.sync.dma_start(
                        out=outq[t, :, z0:z0 + ZB, :],
                        in_=tmp.rearrange("p (z xy) -> p z xy", z=ZB),
                    )

                # interleave: temporal(t) after spatial(t+2), so output
                # DMAs and evacuations spread evenly across the z-block
                # AND temporal's last tap never waits on a slice evac
                # that finished only ~1us earlier
                spatial(0)
                if zb + 1 < NZB:
                    next_slab = load_slab(zb + 1)
                spatial(1)
                for t in range(2, T):
                    spatial(t)
                    temporal(t - 2)
                temporal(T - 2)
                temporal(T - 1)

    nc.finalize()
    return nc


def _prep_inputs(xr, xi, wxyz_r, wxyz_i, wt_r, wt_i):
    xr = np.asarray(xr, np.float32)
    xi = np.asarray(xi, np.float32)

    wsr, wsi = _project(np.asarray(wxyz_r, np.float64), np.asarray(wxyz_i, np.float64), True)
    wtr, wti = _project(np.asarray(wt_r, np.float64), np.asarray(wt_i, np.float64), False)
    wsp = _spatial_lhsT(wsr, wsi)
    wtp = _temporal_lhsT(wtr, wti)

    pads = [(0, 0), (0, 0), (1, 1), (1, 1), (1, 1), (0, 0)]
    xp = np.stack([np.pad(xr, pads, mode="symmetric"),
                   np.pad(xi, pads, mode="symmetric")])  # [ri2, B, T, ZP, YP, XP, C]
    xp = xp.astype(BF16NP)
    in_maps = []
    for core in range(8):
        b, cx = divmod(core, NXC)
        xs = xp[:, b, :, :, :, XC * cx:XC * cx + XI, :]   # [ri2, T, ZP, YP, XI, C]
        xin = np.empty((NR, T, Z, RW), BF16NP)
        for dz in range(KZ):
            for dy in range(KY):
                blk = xs[:, :, dz:dz + Z, dy:dy + Y, :, :]     # [ri,T,Z,Y,XI,C]
                blk = blk.reshape(2, T, Z, 2, 32, XI, C)       # y -> (j, y')
                blk = blk.transpose(6, 0, 1, 2, 3, 5, 4)       # [C,ri,T,Z,j,XI,y']
                blk = blk.reshape(4, T, Z, RW)
                r0 = ((dz * 3 + dy) * 4)
                xin[r0:r0 + 4] = blk
        # [r, zb, t, zp, row] zb-major, split into 2048-elem pieces each
        # stored in its own 4096-elem (two-page) slot, data page-aligned
        def pack(par):                                  # par: 0=even z, 1=odd
            v = xin[:, :, par::2].reshape(NR, T, NZB, 2, RW)
            v = v.transpose(0, 2, 1, 3, 4).reshape(NR, NZB, NPC, 2048)
            out = np.zeros((NR, NZB, NPC, 4096), BF16NP)
            out[:, :, :, 0:2048] = v
            return out

        def pack0(par):                                 # zb0 t-pair chunks
            v = xin[:, :, par:ZB:2].reshape(NR, 4, 2 * 2304)
            return np.ascontiguousarray(v)
        in_maps.append({"xev": pack(0), "xod": pack(1),
                        "xev0": pack0(0), "xod0": pack0(1),
                        "wsp": wsp, "wtp": wtp})
    return in_maps


def kernel(xr, xi, wxyz_r, wxyz_i, wt_r, wt_i):
    if "nc" not in _NC_CACHE:
        _NC_CACHE["nc"] = build_program()
    nc = _NC_CACHE["nc"]

    in_maps = _prep_inputs(xr, xi, wxyz_r, wxyz_i, wt_r, wt_i)
    res = run_bass_kernel_spmd(nc, in_maps, list(range(8)))

    yr = np.empty((B, T, Z, Y, X, F), np.float32)
    yi = np.empty((B, T, Z, Y, X, F), np.float32)
    for core in range(8):
        b, cx = divmod(core, NXC)
        # outq[t, 64j+32q'+f, z, 32x+y'] -> y[t, z, 32j+y', x, f];
        # odd z rows store j swapped
        arr = np.asarray(res.results[core]["outq"], dtype=BF16NP).astype(np.float32)
        arr = arr.transpose(0, 2, 1, 3)               # [t,z,128,512]
        arr = arr.reshape(T, Z, 2, 2, F, XC, 32)      # [t,z,j,q',f,x,y']
        arr[:, 1::2] = arr[:, 1::2, ::-1]
        arr = arr.transpose(0, 1, 2, 6, 5, 4, 3)      # [t,z,j,y',x,f,q']
        arr = arr.reshape(T, Z, Y, XC, F, 2)
        yr[b, :, :, :, XC * cx:XC * cx + XC, :] = arr[..., 0]
        yi[b, :, :, :, XC * cx:XC * cx + XC, :] = arr[..., 1]
    return yr, yi
